# revision 27
# baseline (speedup 1.0000x reference)
"""Trainium2 Bass kernel for a single-head causal attention block.

Reference computation (B=4, T=2048, D=Kd=Vd=1024):
    K = X @ Wk + bk;  Q = X @ Wq + bq;  V = X @ Wv + bv
    S = Q @ K^T / 32, causal-masked;  P = softmax(S);  read = P @ V
    out = concat([X, read], axis=-1)

Sharding: 8 cores = (batch b, query-chunk-pair h).  T is split into 4
chunks of 512; core h=0 owns chunks {0, 3}, core h=1 owns chunks {1, 2}
(1024 queries each, causally load-balanced).  Keys are shipped permuted
into 4 groups of 512:
    G0 = keys of the core's low chunk   (diagonal of query block qc0)
    G1 = keys of the core's high chunk  (diagonal of query block qc1)
    G2 = "restA"  (h=0: rows 512:1024,  h=1: rows 0:512)
    G3 = "restB"  (h=0: rows 1024:1536, h=1: rows 1536:2048)
Under this permutation the mask structure per (s-tile, q-block) is
identical on every core at compile time:
    (G0, qc0) diag-causal | (G0, qc1) visible | (G1, qc0) SKIP |
    (G1, qc1) diag-causal | (G2, qc0) data-bias cbA | (G2, qc1) visible |
    (G3, qc0) SKIP | (G3, qc1) data-bias cbB
cbA/cbB in {0, -1e9} are per-core input data, so the instruction stream
is identical across cores (SPMD) while half-masked work is skipped.

Precision plan (validated by host-side emulation, rel-err ~8.9e-3 vs
the 2e-2 gate): K/Q projections and the T x T attention run in fp8 e4m3
DoubleRow matmuls (2 k-tiles per instruction, 0.5 cycles/row); the V
projection stays bf16 since its error reaches the output unmasked:
    Kt8 [128, 8, T]   = fp8(X@Wk + bk), transposed keys, kd-tile-major
    Qt8 [128, 8, TQ]  = fp8(X@Wq + bq)  (unscaled; 1/32 folded into the
                        exp activation's scale instead, keeping fp8
                        operands well-conditioned)
    S^T = Kt8-pairs @ Qt8-pairs  (4 DoubleRow matmuls per 128x512 tile)
    P^T = exp(S/32 + cbias) stored as fp8 into per-qb tensors
          pt8 [128, ntiles, 512] (zero-padded above the diagonal so PV
          can contract uniform 2-tile pairs)
    V is never materialized: read = P@(X@Wv) = (P@X)@Wv.  Z^T = X^T@P^T
    accumulates in fp8 DoubleRow (X shipped as an exact fp8 hi/lo pair),
    then read = Z@Wv runs in bf16 — this deletes the entire TxDxVD V
    projection.  Row sums of P come from DoubleRow matmuls against a
    ones vector; normalization is folded into the final PSUM evacuation
    as a per-partition reciprocal scale.
    V bias (bv) is added on the host (softmax rows sum to 1).
"""

import sys

for _p in ("/opt/trn_rl_repo", "/root/.axon_site/_ro/trn_rl_repo"):
    if _p not in sys.path:
        sys.path.insert(0, _p)

import numpy as np
import ml_dtypes

N_CORES = 8
P = 128
B, T, D = 4, 2048, 1024
KD, VD = 1024, 1024
TQ = 1024          # queries per core
NDT = D // P       # contraction d-tiles (8)
NMT = KD // P      # d_out tiles for Kt/Qt (8)
NST = T // P       # key s-tiles (16)
NQB = TQ // 512    # q blocks of 512 (2)
NVB = VD // 512    # v blocks of 512 (2)
NEG = -1.0e9

_BF16 = ml_dtypes.bfloat16
_E4M3 = ml_dtypes.float8_e4m3
_CACHE = {}


def _build_nc():
    import concourse.mybir as mybir
    import concourse.tile as tile
    from concourse import bacc

    f32 = mybir.dt.float32
    bf16 = mybir.dt.bfloat16

    nc = bacc.Bacc("TRN2", target_bir_lowering=False, debug=False,
                   num_devices=N_CORES)

    fp8 = mybir.dt.float8e4
    xh_d = nc.dram_tensor("xh", [P, NDT, T], fp8, kind="ExternalInput").ap()
    xrh_d = nc.dram_tensor("xrh", [P, NST, D], fp8, kind="ExternalInput").ap()
    xrl_d = nc.dram_tensor("xrl", [P, NST, D], fp8, kind="ExternalInput").ap()
    wk_d = nc.dram_tensor("wk", [P, NDT, KD], fp8, kind="ExternalInput").ap()
    wq_d = nc.dram_tensor("wq", [P, NDT, KD], fp8, kind="ExternalInput").ap()
    wv_d = nc.dram_tensor("wv", [D, VD], bf16, kind="ExternalInput").ap()
    bk_d = nc.dram_tensor("bkb", [P, NMT], f32, kind="ExternalInput").ap()
    bq_d = nc.dram_tensor("bqb", [P, NMT], f32, kind="ExternalInput").ap()
    cb_d = nc.dram_tensor("cb", [P, 2], f32, kind="ExternalInput").ap()
    out_d = nc.dram_tensor("out", [TQ, VD], f32, kind="ExternalOutput").ap()

    with tile.TileContext(nc) as tc:
        _emit(nc, tc, mybir, xh_d, xrh_d, xrl_d, wk_d, wq_d, wv_d,
              bk_d, bq_d, cb_d, out_d)

    nc.compile()
    return nc


def _emit(nc, tc, mybir, xh_d, xrh_d, xrl_d, wk_d, wq_d, wv_d,
          bk_d, bq_d, cb_d, out_d):
    from contextlib import ExitStack

    f32 = mybir.dt.float32
    bf16 = mybir.dt.bfloat16
    fp8 = mybir.dt.float8e4
    Exp = mybir.ActivationFunctionType.Exp
    DR = mybir.MatmulPerfMode.DoubleRow
    Alu = mybir.AluOpType

    with ExitStack() as ctx:
        constp = ctx.enter_context(tc.tile_pool(name="const", bufs=1))
        xtp = ctx.enter_context(tc.tile_pool(name="xtp", bufs=1))
        wp = ctx.enter_context(tc.tile_pool(name="wp", bufs=1))
        ktp = ctx.enter_context(tc.tile_pool(name="ktp", bufs=1))
        qtp = ctx.enter_context(tc.tile_pool(name="qtp", bufs=1))
        ztp = ctx.enter_context(tc.tile_pool(name="ztp", bufs=1))
        ptp = ctx.enter_context(tc.tile_pool(name="ptp", bufs=1))
        outp = ctx.enter_context(tc.tile_pool(name="outp", bufs=2))
        recp = ctx.enter_context(tc.tile_pool(name="recp", bufs=2))
        proj_ps = ctx.enter_context(
            tc.tile_pool(name="proj_ps", bufs=2, space="PSUM"))
        score_ps = ctx.enter_context(
            tc.tile_pool(name="score_ps", bufs=2, space="PSUM"))
        z_ps = ctx.enter_context(
            tc.tile_pool(name="z_ps", bufs=3, space="PSUM"))
        sum_ps = ctx.enter_context(
            tc.tile_pool(name="sum_ps", bufs=1, space="PSUM"))

        # input loads.  wk (fp8) cols 0:512 + the first xh column block
        # first: the first Kt chains unblock after 1MB of DMA (cuts the
        # startup PE bubble).  K/Q projections contract fp8 DoubleRow
        # pairs; V stays bf16 (its error reaches the output directly,
        # K/Q error is masked by the fp8 score quantization).
        wk8 = wp.tile([P, NDT, KD], fp8)
        xh8 = xtp.tile([P, NDT, T], fp8)
        nc.sync.dma_start(out=wk8[:, :, 0:512], in_=wk_d[:, :, 0:512])
        nc.sync.dma_start(out=xh8[:, :, 0:512], in_=xh_d[:, :, 0:512])
        # constants (needed first at the first PSUM evacuation, ~7us in)
        bk_sb = constp.tile([P, NMT], f32)
        nc.sync.dma_start(out=bk_sb[:], in_=bk_d)
        bq_sb = constp.tile([P, NMT], f32)
        nc.sync.dma_start(out=bq_sb[:], in_=bq_d)
        cb_sb = constp.tile([P, 2], f32)
        nc.sync.dma_start(out=cb_sb[:], in_=cb_d)
        ones2 = constp.tile([P, 2, 1], fp8)
        nc.vector.memset(ones2[:], 1.0)
        nc.sync.dma_start(out=wk8[:, :, 512:KD], in_=wk_d[:, :, 512:KD])
        for cb in range(1, T // 512):
            nc.sync.dma_start(out=xh8[:, :, cb * 512:(cb + 1) * 512],
                              in_=xh_d[:, :, cb * 512:(cb + 1) * 512])
        wq8 = wp.tile([P, NDT, KD], fp8)
        nc.sync.dma_start(out=wq8[:], in_=wq_d)
        xrh = xtp.tile([P, NST, D], fp8)
        nc.sync.dma_start(out=xrh[:], in_=xrh_d)
        xrl = xtp.tile([P, NST, D], fp8)
        nc.sync.dma_start(out=xrl[:], in_=xrl_d)
        wv_sb = []
        for kd in range(NDT):
            wtile = wp.tile([P, KD], bf16, name=f"wv{kd}")
            nc.sync.dma_start(out=wtile[:], in_=wv_d[kd * P:(kd + 1) * P, :])
            wv_sb.append(wtile)

        # ---- projections ----
        # Kt8[:, m, s] = fp8(sum_d X[s, d] Wk[d, m*128+p] + bk), computed
        # as fp8(X) @ fp8(Wk) DoubleRow pairs (error masked by the fp8
        # score-input quantization).
        # nb-outer so the first 8 chains all depend only on x column block 0
        kt8 = ktp.tile([P, NMT, T], fp8)
        for nb in range(T // 512):
            for m in range(NMT):
                ps = proj_ps.tile([P, 512], f32, name="proj")
                for j in range(NDT // 2):
                    nc.tensor.matmul(
                        ps[:],
                        lhsT=wk8[:, 2 * j:2 * j + 2, m * P:(m + 1) * P],
                        rhs=xh8[:, 2 * j:2 * j + 2,
                                nb * 512:(nb + 1) * 512],
                        start=(j == 0), stop=(j == NDT // 2 - 1),
                        perf_mode=DR)
                nc.vector.tensor_scalar_add(
                    out=kt8[:, m, nb * 512:(nb + 1) * 512],
                    in0=ps[:], scalar1=bk_sb[:, m:m + 1])

        # Qt8 (queries are the first TQ permuted columns of x; unscaled)
        qt8 = qtp.tile([P, NMT, TQ], fp8)
        for m in range(NMT):
            for qb in range(NQB):
                ps = proj_ps.tile([P, 512], f32, name="proj")
                for j in range(NDT // 2):
                    nc.tensor.matmul(
                        ps[:],
                        lhsT=wq8[:, 2 * j:2 * j + 2, m * P:(m + 1) * P],
                        rhs=xh8[:, 2 * j:2 * j + 2,
                                qb * 512:(qb + 1) * 512],
                        start=(j == 0), stop=(j == NDT // 2 - 1),
                        perf_mode=DR)
                nc.vector.tensor_scalar_add(
                    out=qt8[:, m, qb * 512:(qb + 1) * 512],
                    in0=ps[:], scalar1=bq_sb[:, m:m + 1])

        # V is never materialized: read = P@V = (P@X)@Wv (reassociation).
        # Z^T[d, q] = X^T @ P^T accumulates in fp8 DoubleRow over key-tile
        # pairs (X shipped as an exact fp8 hi/lo pair), then read = Z@Wv in
        # bf16.  This deletes the whole T x D x VD V projection.
        zt = ztp.tile([P, NDT, TQ], bf16)

        # ---- attention, one 512-wide query block at a time ----
        # tile kind per (qc, s-tile): "diag" (affine_select; only columns
        # >= the tile's diagonal start are computed), "vis" (no mask),
        # "cbA"/"cbB" (per-core data bias), or skipped (always masked)
        def tile_kind(qc, st):
            g = st // 4
            if qc == 0:
                return ("diag", st * P) if g == 0 else \
                       ("cbA", 0) if g == 2 else None
            return ("vis", 0) if g in (0, 2) else \
                   ("diag", (st - 4) * P) if g == 1 else ("cbB", 0)

        # per-qb P tensors: dim1 slot -> s-tile (in ascending st order of
        # the computed tiles), zero-padded above the diagonal
        qb_sts = [[st for st in range(NST) if tile_kind(qb, st)]
                  for qb in range(NQB)]
        pt8 = [ptp.tile([P, len(qb_sts[qb]), 512], fp8, name=f"pt8_{qb}")
               for qb in range(NQB)]

        for qb in range(NQB):
            slot_of = {st: i for i, st in enumerate(qb_sts[qb])}
            for st in qb_sts[qb]:
                kname, off = tile_kind(qb, st)
                ncols = 512 - off
                slot = slot_of[st]
                if off:
                    # zero the uncomputed left part so PV pairs can
                    # contract the full 512 columns uniformly
                    nc.vector.memset(pt8[qb][:, slot, 0:off], 0.0)
                ps = score_ps.tile([P, ncols], f32, name="score")
                for j in range(NMT // 2):
                    nc.tensor.matmul(
                        ps[:],
                        lhsT=kt8[:, 2 * j:2 * j + 2, st * P:(st + 1) * P],
                        rhs=qt8[:, 2 * j:2 * j + 2,
                                qb * 512 + off:(qb + 1) * 512],
                        start=(j == 0), stop=(j == NMT // 2 - 1),
                        perf_mode=DR)
                bias = 0.0
                if kname == "cbA":
                    bias = cb_sb[:, 0:1]
                elif kname == "cbB":
                    bias = cb_sb[:, 1:2]
                pt_out = pt8[qb][:, slot, off:512]
                nc.scalar.activation(out=pt_out, in_=ps[:], func=Exp,
                                     bias=bias, scale=1.0 / 32.0)
                if kname == "diag":
                    # zero the strictly-upper (key>query) part; in local
                    # coords the diagonal starts at column 0
                    nc.gpsimd.affine_select(
                        out=pt_out, in_=pt_out,
                        compare_op=mybir.AluOpType.is_ge, fill=0.0,
                        base=0, channel_multiplier=-1,
                        pattern=[[1, ncols]])

        for qb in range(NQB):
            # Z^T accumulation: for each d-tile, Z^T[d, qb-block] =
            # sum over key-tile pairs of (Xh|Xl)-pair^T @ P^T-pair
            # (DoubleRow).  Zero-padded P regions contribute nothing, so
            # the pair list is uniform across the 512-q block.  d-tiles
            # run in waves of 3 (z_ps has 3 PSUM banks).
            npairs = len(qb_sts[qb]) // 2
            for wave in range((NDT + 2) // 3):
                dts = range(3 * wave, min(3 * wave + 3, NDT))
                zps = {dt: z_ps.tile([P, 512], f32, name="zps")
                       for dt in dts}
                for dt in dts:
                    nmm = 2 * npairs
                    n = 0
                    for i in range(npairs):
                        sp0 = qb_sts[qb][2 * i]
                        assert qb_sts[qb][2 * i + 1] == sp0 + 1
                        rhs = pt8[qb][:, 2 * i:2 * i + 2, :]
                        for xr in (xrh, xrl):
                            nc.tensor.matmul(
                                zps[dt][:],
                                lhsT=xr[:, sp0:sp0 + 2,
                                        dt * P:(dt + 1) * P],
                                rhs=rhs,
                                start=(n == 0), stop=(n == nmm - 1),
                                perf_mode=DR)
                            n += 1
                for dt in dts:
                    nc.vector.tensor_scalar_add(
                        out=zt[:, dt, qb * 512:(qb + 1) * 512],
                        in0=zps[dt][:], scalar1=0.0)

            # per q-tile: row sums of P (for softmax normalization) and
            # read = Z @ Wv (bf16), normalized during PSUM evacuation
            nslots = len(qb_sts[qb])
            for qtl in range(4):
                pairs = []
                for i in range(nslots // 2):
                    st0 = qb_sts[qb][2 * i]
                    kind0 = tile_kind(qb, st0)
                    if kind0[0] == "diag" and kind0[1] > qtl * P:
                        continue  # both tiles strictly above diagonal
                    pairs.append(i)
                sums = sum_ps.tile([P, 1], f32, name="sums")
                npair = len(pairs)
                for n, i in enumerate(pairs):
                    lhsT = pt8[qb][:, 2 * i:2 * i + 2,
                                   qtl * P:(qtl + 1) * P]
                    nc.tensor.matmul(sums[:], lhsT=lhsT, rhs=ones2[:],
                                     start=(n == 0), stop=(n == npair - 1),
                                     perf_mode=DR)
                qt_g = qb * 4 + qtl
                q0 = qb * 512 + qtl * P
                pvs = [score_ps.tile([P, 512], f32, name="score")
                       for _ in range(NVB)]
                for dt in range(NDT):
                    lhsT = zt[:, dt, q0:q0 + P]
                    for vb in range(NVB):
                        nc.tensor.matmul(
                            pvs[vb][:], lhsT=lhsT,
                            rhs=wv_sb[dt][:, vb * 512:(vb + 1) * 512],
                            start=(dt == 0), stop=(dt == NDT - 1))
                recip = recp.tile([P, 1], f32, name="recip")
                nc.vector.reciprocal(out=recip[:], in_=sums[:])
                ob = outp.tile([P, VD], f32, name="ob")
                for vb in range(NVB):
                    # on DVE, not ACT: ACT is busy with the exp stream;
                    # store each 512-block as soon as it is normalized
                    nc.vector.tensor_scalar_mul(
                        out=ob[:, vb * 512:(vb + 1) * 512], in0=pvs[vb][:],
                        scalar1=recip[:, 0:1])
                    nc.sync.dma_start(
                        out=out_d[qt_g * P:(qt_g + 1) * P,
                                  vb * 512:(vb + 1) * 512],
                        in_=ob[:, vb * 512:(vb + 1) * 512])


def _install_neff_disk_cache():
    """Wrap libneuronxla.neuronx_cc with a content-hash disk cache so
    identical kernels skip the multi-minute walrus compile across
    processes."""
    import hashlib
    import os
    import pickle

    try:
        import libneuronxla
    except ImportError:
        return
    if getattr(libneuronxla, "_bass_neff_cache_installed", False):
        return
    try:
        cache_dir = os.path.expanduser("~/.bass_neff_cache")
        os.makedirs(cache_dir, exist_ok=True)
    except Exception:
        return
    inner = libneuronxla.neuronx_cc

    def cached_cc(code, code_format, platform_version, file_prefix):
        key = hashlib.sha256(
            b"%s|%s|%s" % (bytes(code), bytes(code_format),
                           str(platform_version).encode())
        ).hexdigest()
        path = os.path.join(cache_dir, key + ".pkl")
        if os.path.exists(path):
            try:
                with open(path, "rb") as f:
                    return pickle.load(f)
            except Exception:
                pass
        result = inner(code, code_format, platform_version, file_prefix)
        try:
            tmp = path + ".tmp.%d" % os.getpid()
            with open(tmp, "wb") as f:
                pickle.dump(result, f)
            os.replace(tmp, path)
        except Exception:
            pass
        return result

    libneuronxla.neuronx_cc = cached_cc
    libneuronxla._bass_neff_cache_installed = True


def _make_runner(nc):
    """Build a cached jitted SPMD runner (mirrors bass2jax.run_bass_via_pjrt
    but reuses one jax.jit across calls)."""
    import jax
    import concourse.mybir as mybir
    from concourse import bass2jax
    from jax.sharding import Mesh, PartitionSpec
    try:
        from jax.experimental.shard_map import shard_map
    except ImportError:
        from jax.shard_map import shard_map

    bass2jax.install_neuronx_cc_hook()
    _install_neff_disk_cache()
    assert nc.dbg_addr is None
    partition_name = (nc.partition_id_tensor.name
                      if nc.partition_id_tensor else None)

    in_names, out_names, out_avals, zero_shapes = [], [], [], []
    for alloc in nc.m.functions[0].allocations:
        if not isinstance(alloc, mybir.MemoryLocationSet):
            continue
        name = alloc.memorylocations[0].name
        if alloc.kind == "ExternalInput":
            if name != partition_name:
                in_names.append(name)
        elif alloc.kind == "ExternalOutput":
            shape = tuple(alloc.tensor_shape)
            dtype = mybir.dt.np(alloc.dtype)
            out_names.append(name)
            out_avals.append(jax.core.ShapedArray(shape, dtype))
            zero_shapes.append((shape, dtype))
    n_params = len(in_names)
    all_names = in_names + out_names
    if partition_name is not None:
        all_names = all_names + [partition_name]
    donate = tuple(range(n_params, n_params + len(out_names)))

    def _body(*args):
        operands = list(args)
        if partition_name is not None:
            operands.append(bass2jax.partition_id_tensor())
        outs = bass2jax._bass_exec_p.bind(
            *operands,
            out_avals=tuple(out_avals),
            in_names=tuple(all_names),
            out_names=tuple(out_names),
            lowering_input_output_aliases=(),
            sim_require_finite=True,
            sim_require_nnan=True,
            nc=nc,
        )
        return tuple(outs)

    devices = jax.devices()[:N_CORES]
    assert len(devices) == N_CORES, f"need {N_CORES} cores, have {len(jax.devices())}"
    mesh = Mesh(np.asarray(devices), ("core",))
    n_args = n_params + len(out_names)
    sharded = jax.jit(
        shard_map(_body, mesh=mesh,
                  in_specs=(PartitionSpec("core"),) * n_args,
                  out_specs=(PartitionSpec("core"),) * len(out_names),
                  check_rep=False),
        donate_argnums=donate, keep_unused=True)

    def run(in_maps):
        concat_in = [
            np.concatenate([np.asarray(m[name]) for m in in_maps], axis=0)
            for name in in_names
        ]
        concat_zeros = [
            np.zeros((N_CORES * s[0], *s[1:]), dt) for s, dt in zero_shapes
        ]
        out_arrs = sharded(*concat_in, *concat_zeros)
        out_arrs = [np.asarray(a) for a in out_arrs]
        return [
            {name: out_arrs[i].reshape(N_CORES, *out_avals[i].shape)[c]
             for i, name in enumerate(out_names)}
            for c in range(N_CORES)
        ]

    return run


def _get_runner():
    if "runner" not in _CACHE:
        nc = _build_nc()
        _CACHE["nc"] = nc
        _CACHE["runner"] = _make_runner(nc)
    return _CACHE["runner"]


def _dr_layout(a, ncols):
    """[D, N] -> [P, D//P, N] DoubleRow layout (contraction d = kd*128 + p)."""
    return np.ascontiguousarray(
        a.reshape(NDT, P, ncols).transpose(1, 0, 2))


def _prep_in_maps(inputs, Wk, bk, Wq, bq, Wv, bv):
    f32 = np.float32
    wk_8 = _dr_layout(np.ascontiguousarray(Wk, dtype=f32).astype(_E4M3), KD)
    wq_8 = _dr_layout(np.ascontiguousarray(Wq, dtype=f32).astype(_E4M3), KD)
    wv_b = np.ascontiguousarray(Wv, dtype=f32).astype(_BF16)
    bkb = np.ascontiguousarray(bk.reshape(NMT, P).T, dtype=f32)
    bqb = np.ascontiguousarray(bq.reshape(NMT, P).T, dtype=f32)
    in_maps = []
    for c in range(N_CORES):
        b, h = c // 2, c % 2
        Xb = inputs[b]
        if h == 0:
            # chunks {0, 3}: G0=rows 0:512, G1=1536:2048, G2=512:1024,
            # G3=1024:1536; cbA=-1e9 (G2 after chunk0's queries), cbB=0
            perm = np.r_[0:512, 1536:2048, 512:1024, 1024:1536]
            cbA, cbB = NEG, 0.0
        else:
            # chunks {1, 2}: G0=rows 512:1024, G1=1024:1536, G2=0:512,
            # G3=1536:2048; cbA=0 (G2 before chunk1), cbB=-1e9
            perm = np.r_[512:1024, 1024:1536, 0:512, 1536:2048]
            cbA, cbB = 0.0, NEG
        Xp = np.ascontiguousarray(Xb[perm])  # [T, D] f32, permuted rows
        xt = np.ascontiguousarray(Xp.T)      # [D, T]
        xh = xt.astype(_E4M3)
        xrh = Xp.astype(_E4M3)
        xrl = (Xp - xrh.astype(f32)).astype(_E4M3)
        cb = np.empty((P, 2), dtype=f32)
        cb[:, 0] = cbA
        cb[:, 1] = cbB
        in_maps.append({
            "xh": _dr_layout(xh, T),
            "xrh": np.ascontiguousarray(
                xrh.reshape(NST, P, D).transpose(1, 0, 2)),
            "xrl": np.ascontiguousarray(
                xrl.reshape(NST, P, D).transpose(1, 0, 2)),
            "wk": wk_8, "wq": wq_8, "wv": wv_b,
            "bkb": bkb, "bqb": bqb, "cb": cb,
        })
    return in_maps


def kernel(inputs, Wk, bk, Wq, bq, Wv, bv):
    inputs = np.asarray(inputs, dtype=np.float32)
    run = _get_runner()
    in_maps = _prep_in_maps(inputs, Wk, bk, Wq, bq, Wv, bv)
    results = run(in_maps)
    bvf = np.asarray(bv, dtype=np.float32)
    read = np.empty((B, T, VD), dtype=np.float32)
    for c in range(N_CORES):
        b, h = c // 2, c % 2
        out_c = results[c]["out"] + bvf
        if h == 0:
            read[b, 0:512] = out_c[0:512]        # chunk 0
            read[b, 1536:2048] = out_c[512:1024]  # chunk 3
        else:
            read[b, 512:1024] = out_c[0:512]      # chunk 1
            read[b, 1024:1536] = out_c[512:1024]  # chunk 2
    return np.concatenate([inputs, read], axis=2)

